# revision 41
# baseline (speedup 1.0000x reference)
"""BiasedMHA Trainium2 kernel (v2).

Problem: B=4, N=1024, FEAT=512, H=8 multihead attention with additive bias and
boolean mask, softmax over the key dim, output projection.

Sharding (8 cores): core c handles batch b = c//2 and head-group hg = c%2
(4 heads), over ALL 1024 queries and keys. Each core emits an UNNORMALIZED
output-projection partial (its 4 heads' contribution); the host unshard sums
the two partials per batch and adds the constant row bo + bv@Wo.T. Versus the
old (batch, query-half) split this removes the duplicated K/V projections.

Per-core math:
  - k-bias bk is dropped: scores_ij = q_i.(k_j + bk) differs from q_i.k_j by a
    per-row constant, which softmax cancels exactly.
  - exp(scores + bias) is factored as exp(scores) * expb with
    expb = where(mask, 0, exp(bias)) precomputed on host in f16 (masked
    entries become exactly 0). ACT does exp straight out of PSUM; the DVE does
    one f16 multiply (2x mode) instead of an f32 PSUM add.
  - PV matmuls carry a ones-column per head (row 64 = softmax denominator).
    1/denom = exp(-ln(denom)) on ACT (both funcs live in the one pinned
    table set), broadcast down 64 partitions with a K=1 ones matmul.
  - Projections/scores/PV all f16 (fp8 fails the 2e-2 absmax gate; measured
    3.2e-2 with fp8 q/k alone).
  - Head pairs interleave through the score->exp->mult->PV pipeline so the
    PE stays busy enough to hold its fast p-state.
"""

import numpy as np

import concourse.bass as bass
import concourse.mybir as mybir
import concourse.tile as tile
from concourse import bacc
from concourse.bass_utils import run_bass_kernel_spmd

# Pin the activation pass to the exp table set so exactly one ACT_TABLE_LOAD
# is emitted (Exp is the only ACT function used).
_orig_get_tables = bacc.get_activation_tables


def _one_table(arch):
    t = _orig_get_tables(arch)
    return {k: (v if k == "natural_log_exp_and_others" else set())
            for k, v in t.items()}


bacc.get_activation_tables = _one_table

B, N, FEAT, H = 4, 1024, 512, 8
HD = FEAT // H          # 64
SCALE = HD ** -0.5
N_CORES = 8
HL = 4                  # local heads per core
NJC = N // 128          # 8 key chunks
NFC = FEAT // 128       # 4 contraction chunks

F32 = mybir.dt.float32
F16 = mybir.dt.float16
AF = mybir.ActivationFunctionType

_CACHE = {}


def _build():
    nc = bacc.Bacc("TRN2", target_bir_lowering=False, debug=False)

    ndT = nc.dram_tensor("ndT", [FEAT, N], F16, kind="ExternalInput").ap()
    wk2 = nc.dram_tensor("wk2", [FEAT, 256], F16, kind="ExternalInput").ap()
    wq2 = nc.dram_tensor("wq2", [FEAT, 256], F16, kind="ExternalInput").ap()
    wv2 = nc.dram_tensor("wv2", [FEAT, 256], F16, kind="ExternalInput").ap()
    wo2 = nc.dram_tensor("wo2", [256, FEAT], F16, kind="ExternalInput").ap()
    bq2 = nc.dram_tensor("bq2", [128, 2], F32, kind="ExternalInput").ap()
    expb = nc.dram_tensor("expb", [HL, 128, NJC * N], F16,
                          kind="ExternalInput").ap()
    out = nc.dram_tensor("out", [N, FEAT], F32, kind="ExternalOutput").ap()

    with tile.TileContext(nc) as tc:
        with (
            tc.tile_pool(name="persist", bufs=1) as persist,
            tc.tile_pool(name="est", bufs=3) as est_pool,
            tc.tile_pool(name="etp", bufs=4) as et_pool,
            tc.tile_pool(name="tailp", bufs=2) as tailp,
            tc.tile_pool(name="outcp", bufs=2) as outcp,
            tc.tile_pool(name="ps_st", bufs=3, space="PSUM") as ps_st,
            tc.tile_pool(name="ps_a", bufs=4, space="PSUM") as ps_a,
            tc.tile_pool(name="ps_rb", bufs=1, space="PSUM") as ps_rb,
        ):
            nd = [persist.tile([128, N], F16, tag=f"nd{fc}", name=f"nd{fc}")
                  for fc in range(NFC)]
            wk = [persist.tile([128, 256], F16, tag=f"wk{fc}", name=f"wk{fc}")
                  for fc in range(NFC)]
            wq = [persist.tile([128, 256], F16, tag=f"wq{fc}", name=f"wq{fc}")
                  for fc in range(NFC)]
            wv = [persist.tile([128, 256], F16, tag=f"wv{fc}", name=f"wv{fc}")
                  for fc in range(NFC)]
            wo = [persist.tile([64, FEAT], F16, tag=f"wo{h}", name=f"wo{h}")
                  for h in range(HL)]
            KT = [persist.tile([128, N], F16, tag=f"kt{t}", name=f"kt{t}")
                  for t in range(2)]
            QT = [persist.tile([128, N], F16, tag=f"qt{t}", name=f"qt{t}")
                  for t in range(2)]
            V = persist.tile([128, NJC * (HL * 65)], F16, tag="v", name="v")
            OTn = [persist.tile([64, N], F16, tag=f"otn{h}", name=f"otn{h}")
                   for h in range(HL)]
            bq_sb = persist.tile([128, 2], F32, tag="bq")
            ones_sb = persist.tile([128, 64], F16, tag="ones")
            eb = [persist.tile([128, NJC * N], F16, tag=f"eb{h}",
                               name=f"eb{h}")
                  for h in range(HL)]

            # input DMAs in consumption order
            nc.sync.dma_start(out=bq_sb, in_=bq2)
            for fc in range(NFC):
                nc.sync.dma_start(out=wk[fc],
                                  in_=wk2[fc * 128:(fc + 1) * 128, :])
                nc.sync.dma_start(out=nd[fc],
                                  in_=ndT[fc * 128:(fc + 1) * 128, :])
            for fc in range(NFC):
                nc.sync.dma_start(out=wq[fc],
                                  in_=wq2[fc * 128:(fc + 1) * 128, :])
                nc.sync.dma_start(out=wv[fc],
                                  in_=wv2[fc * 128:(fc + 1) * 128, :])
            nc.sync.dma_start(out=eb[0], in_=expb[0])
            nc.sync.dma_start(out=eb[1], in_=expb[1])
            for h in range(HL):
                nc.sync.dma_start(out=wo[h],
                                  in_=wo2[h * 64:(h + 1) * 64, :])
            nc.sync.dma_start(out=eb[2], in_=expb[2])
            nc.sync.dma_start(out=eb[3], in_=expb[3])

            nc.gpsimd.memset(ones_sb, 1.0)
            # PE p-state warmup: chain dummy matmuls (no DMA deps) through
            # the tail's PSUM bank while the first input DMAs land, so the
            # projection matmuls start at full clock instead of ramping.
            warm = persist.tile([64, 512], F16, tag="warm")
            nc.gpsimd.memset(warm, 0.0)
            for i in range(10):
                wps = ps_rb.tile([128, 512], F32, tag="rbc", name=f"warm{i}")
                nc.tensor.matmul(wps[0:64, :], warm[:, 0:64], warm,
                                 start=True, stop=True)
            # ones columns of V: per (jc, h) column 64 within the 65-block
            nc.gpsimd.memset(
                V.rearrange("p (jc h x) -> p jc h x", h=HL, x=65)[:, :, :, 64:65],
                1.0,
            )

            # ---- projections ----
            def emit_kproj(t):
                for run in range(2):
                    ps = ps_a.tile([128, 512], F32, tag="mm",
                                   name=f"kp{t}{run}")
                    for fc in range(NFC):
                        nc.tensor.matmul(
                            ps,
                            wk[fc][:, t * 128:(t + 1) * 128],
                            nd[fc][:, run * 512:(run + 1) * 512],
                            start=(fc == 0), stop=(fc == NFC - 1),
                        )
                    nc.vector.tensor_copy(
                        KT[t][:, run * 512:(run + 1) * 512], ps)

            def emit_qproj(t):
                for run in range(2):
                    ps = ps_a.tile([128, 512], F32, tag="mm",
                                   name=f"qp{t}{run}")
                    for fc in range(NFC):
                        nc.tensor.matmul(
                            ps,
                            wq[fc][:, t * 128:(t + 1) * 128],
                            nd[fc][:, run * 512:(run + 1) * 512],
                            start=(fc == 0), stop=(fc == NFC - 1),
                        )
                    nc.vector.tensor_scalar_add(
                        QT[t][:, run * 512:(run + 1) * 512], ps,
                        bq_sb[:, t:t + 1])

            def emit_vproj(jt):
                ps = ps_a.tile([128, 512], F32, tag="mm", name=f"vp{jt}")
                for fc in range(NFC):
                    nc.tensor.matmul(
                        ps[:, 0:256],
                        nd[fc][:, jt * 128:(jt + 1) * 128],
                        wv[fc],
                        start=(fc == 0), stop=(fc == NFC - 1),
                    )
                nc.vector.tensor_copy(
                    V.rearrange("p (jc h x) -> p jc h x", h=HL, x=65)
                     [:, jt, :, 0:64],
                    ps.rearrange("p (h x) -> p h x", x=64)[:, 0:HL, :],
                )

            # ---- attention for one head pair, interleaved for PE overlap ----
            def emit_pair(t):
                heads = (2 * t, 2 * t + 1)
                pv = {h: [ps_a.tile([128, 512], F32, tag="mm",
                                    name=f"pv{h}{r}")
                          for r in range(2)]
                      for h in heads}
                for jc in range(NJC):
                    for h in heads:
                        po = 64 * (h % 2)
                        e_st = est_pool.tile([128, 1024], F16, tag="est",
                                             name=f"es{h}_{jc}")
                        for run in range(2):
                            st = ps_st.tile([128, 512], F32, tag="st",
                                            name=f"st{h}_{jc}_{run}")
                            nc.tensor.matmul(
                                st,
                                KT[t][po:po + 64, jc * 128:(jc + 1) * 128],
                                QT[t][po:po + 64, run * 512:(run + 1) * 512],
                                start=True, stop=True,
                            )
                            nc.scalar.activation(
                                e_st[:, run * 512:(run + 1) * 512], st, AF.Exp)
                        et = et_pool.tile([128, 1024], F16, tag="et",
                                          name=f"et{h}_{jc}")
                        nc.vector.tensor_mul(
                            et, e_st, eb[h][:, jc * 1024:(jc + 1) * 1024])
                        for run in range(2):
                            nc.tensor.matmul(
                                pv[h][run][0:65, :],
                                V[:, jc * 260 + h * 65:jc * 260 + h * 65 + 65],
                                et[:, run * 512:(run + 1) * 512],
                                start=(jc == 0), stop=(jc == NJC - 1),
                            )
                # tail: 1/denom = exp(-ln(denom)) on ACT, K=1 matmul
                # broadcast down 64 partitions, then normalize on DVE
                for h in heads:
                    lnr = tailp.tile([65, 1024], F32, tag="lnr",
                                     name=f"lnr{h}")
                    for run in range(2):
                        nc.scalar.activation(
                            lnr[64:65, run * 512:(run + 1) * 512],
                            pv[h][run][64:65, :], AF.Ln)
                    rec = tailp.tile([65, 1024], F16, tag="rec",
                                     name=f"rec{h}")
                    nc.scalar.activation(rec[64:65, :], lnr[64:65, :], AF.Exp,
                                         scale=-1.0)
                    for run in range(2):
                        rbc = ps_rb.tile([128, 512], F32, tag="rbc",
                                         name=f"rbc{h}{run}")
                        nc.tensor.matmul(
                            rbc[0:64, :], ones_sb[64:65, :],
                            rec[64:65, run * 512:(run + 1) * 512],
                            start=True, stop=True,
                        )
                        otd = tailp.tile([64, 512], F32, tag="otd",
                                         name=f"otd{h}{run}")
                        nc.vector.tensor_copy(otd, pv[h][run][0:64, :])
                        nc.vector.tensor_mul(
                            OTn[h][:, run * 512:(run + 1) * 512],
                            otd, rbc[0:64, :])

            emit_kproj(0)
            emit_qproj(0)
            for jt in range(4):
                emit_vproj(jt)
            emit_kproj(1)
            emit_qproj(1)
            for jt in range(4, NJC):
                emit_vproj(jt)
            emit_pair(0)
            emit_pair(1)

            # ---- output projection (4-head partial; host adds the rest) ----
            for it in range(N // 128):
                fp = ps_a.tile([128, 512], F32, tag="mm", name=f"fp{it}")
                for h in range(HL):
                    nc.tensor.matmul(
                        fp,
                        OTn[h][:, it * 128:(it + 1) * 128],
                        wo[h],
                        start=(h == 0), stop=(h == HL - 1),
                    )
                fcp = outcp.tile([128, 512], F32, tag="fcp", name=f"fcp{it}")
                nc.vector.tensor_copy(fcp, fp)
                nc.sync.dma_start(out=out[it * 128:(it + 1) * 128, :], in_=fcp)

    nc.compile()
    return nc


def _prep_inputs(ndata, attn_bias, attn_mask, Wq, bq, Wk, bk, Wv, bv, Wo, bo):
    ndata = np.asarray(ndata, dtype=np.float32)
    attn_bias = np.asarray(attn_bias, dtype=np.float32)
    attn_mask = np.asarray(attn_mask)
    Wq, Wk, Wv, Wo = (np.asarray(w, dtype=np.float32) for w in (Wq, Wk, Wv, Wo))
    bq, bv, bo = (np.asarray(v, dtype=np.float32) for v in (bq, bv, bo))

    # exp(bias) with the mask folded in as exact zeros
    ebf = np.where(attn_mask, np.float32(0.0),
                   np.exp(attn_bias)).astype(np.float16)  # [B, N(i), N(j), H]

    wqT = np.ascontiguousarray((Wq.T * SCALE).astype(np.float16))
    wkT = np.ascontiguousarray(Wk.T.astype(np.float16))
    wvT = np.ascontiguousarray(Wv.T.astype(np.float16))
    woT = np.ascontiguousarray(Wo.T.astype(np.float16))

    in_maps = []
    for core in range(N_CORES):
        b, hg = core // 2, core % 2
        h0 = hg * HL
        # expb layout: [h_local, 128(p=j%128), jc*1024 + i]
        a = ebf[b, :, :, h0:h0 + HL]          # [1024(i), 1024(j), 4]
        a = a.transpose(2, 1, 0)              # [4, 1024(j), 1024(i)]
        a = a.reshape(HL, NJC, 128, N)        # [4, jc, p, i]
        a = a.transpose(0, 2, 1, 3)           # [4, p, jc, i]
        a = np.ascontiguousarray(a.reshape(HL, 128, NJC * N))
        bq2 = np.ascontiguousarray(
            (bq[h0 * HD:(h0 + HL) * HD] * SCALE).reshape(2, 128).T
        ).astype(np.float32)
        in_maps.append({
            "ndT": np.ascontiguousarray(ndata[b].T.astype(np.float16)),
            "wk2": np.ascontiguousarray(wkT[:, h0 * HD:(h0 + HL) * HD]),
            "wq2": np.ascontiguousarray(wqT[:, h0 * HD:(h0 + HL) * HD]),
            "wv2": np.ascontiguousarray(wvT[:, h0 * HD:(h0 + HL) * HD]),
            "wo2": np.ascontiguousarray(woT[h0 * HD:(h0 + HL) * HD, :]),
            "bq2": bq2,
            "expb": a,
        })
    boe = (bo + bv @ Wo.T).astype(np.float32)
    return in_maps, boe


def kernel(ndata, attn_bias, attn_mask, Wq, bq, Wk, bk, Wv, bv, Wo, bo,
           _trace=False):
    if "nc" not in _CACHE:
        _CACHE["nc"] = _build()
    nc = _CACHE["nc"]
    in_maps, boe = _prep_inputs(ndata, attn_bias, attn_mask, Wq, bq, Wk, bk,
                                Wv, bv, Wo, bo)
    res = run_bass_kernel_spmd(nc, in_maps, list(range(N_CORES)), trace=_trace)
    _CACHE["last_res"] = res
    full = np.empty((B, N, FEAT), dtype=np.float32)
    for b in range(B):
        full[b] = (res.results[2 * b]["out"] + res.results[2 * b + 1]["out"]
                   + boe[None, :])
    return full
